# revision 24
# baseline (speedup 1.0000x reference)
"""MoE (cosine-routed, top-k, 2-layer GELU FFN) on 8 Trainium2 NeuronCores.

Strategy (expert-parallel, per the sharding hint):
  - Host computes the (tiny) routing: cosine scores -> softmax -> top-k ->
    renormalized gate weights. ~34 MFLOP, negligible vs the 34 GFLOP FFN.
  - Tokens are dispatched by top-k expert id: core e receives the tokens
    routed to expert e (padded to capacity C), plus expert e's W1/b1/W2/b2.
  - Each core runs the 2-layer FFN in bf16 (fp32 PSUM accumulation) and
    scales each token's output by its gate weight on-device.
  - Host scatter-adds the (<= top_k) expert contributions per token.

Device layout per core (P = 128 partitions):
  GEMM1: hT[f, t] = sum_d W1[d, f] * xT[d, t]   (W1 tiles stationary)
         -> Gelu(. + b1) on ScalarE, cast to bf16
  GEMM2: yT[d, t] = sum_f W2[f, d] * hT[f, t]   (W2 tiles stationary)
         -> (. + b2) * gate on VectorE, bf16 out

Pipeline (v2):
  - x is DMA'd in 4 d-pair blocks at the head of the Sync HWDGE ring;
    GEMM1's first f-block accumulates d-blocks in arrival order, so the
    PE starts ~2 us after the preamble and never idles long enough for
    the HAM clock gate to re-throttle.
  - W1 f0..f7 (+ meta) stream on the Scalar HWDGE ring concurrently with
    x on the Sync ring; W1 f8..f15 and W2 follow x on the Sync ring.
  - PSUM: GEMM1 chunk-0 triple-buffered so GELU drains can lag ~2 f-blocks
    without stalling the PE; all 8 banks used.
  - Outputs are written bf16 (host combines in fp32), one DMA per
    128-row output block.
"""

import numpy as np
import ml_dtypes

P = 128
D_MODEL = 1024
D_FF = 2048
N_EXPERTS = 8
N_CORES = 8
N_WARMUP_MM = 13

_BF16 = ml_dtypes.bfloat16

_cache: dict = {}
last_results = None  # BassKernelResults of the most recent run (for profiling)


def _chunks(C):
    out = []
    c0 = 0
    while c0 < C:
        cw = min(512, C - c0)
        out.append((c0, cw))
        c0 += cw
    return out


def _build(C):
    """Build + compile the SPMD FFN kernel for capacity C (multiple of 32)."""
    import concourse.mybir as mybir
    from concourse import bacc
    from concourse.tile import TileContext

    D, F = D_MODEL, D_FF
    ND, NF = D // P, F // P

    nc = bacc.Bacc("TRN2", target_bir_lowering=False, debug=False,
                   enable_partition_id=False)

    # Host-pre-arranged layouts (see kernel() for the packing):
    #   xT:  [P, ND*C]    column d*C + t       = x[token t, d*P + part]
    #   w1:  [P, NF*ND*P] column f*ND*P + d*P + j = W1[d*P + part, f*P + j]
    #   w2:  [P, NF*D]    column f*D + j       = W2[f*P + part, j]
    xT_d = nc.dram_tensor("xT", [P, ND * C], mybir.dt.bfloat16, kind="ExternalInput")
    w1_d = nc.dram_tensor("w1", [P, NF * ND * P], mybir.dt.bfloat16,
                          kind="ExternalInput")
    w2_d = nc.dram_tensor("w2", [P, NF * D], mybir.dt.bfloat16, kind="ExternalInput")
    meta_d = nc.dram_tensor("meta", [P, NF + ND + C], mybir.dt.float32,
                            kind="ExternalInput")
    out_d = nc.dram_tensor("out", [D, C], mybir.dt.bfloat16, kind="ExternalOutput")

    ck = _chunks(C)
    # PSUM bank budget (8 banks).  Two-chunk: GEMM1 chunk0 x3, chunk1 x2,
    # GEMM2 chunk0 x2, chunk1 x1.  Single-chunk (C <= 512): GEMM1 x4,
    # GEMM2 x3.  Warm-up accumulator shares the GEMM2 slot (disjoint
    # lifetime).
    if len(ck) == 1:
        ps1bufs = lambda ci: 4
        ps2bufs = lambda ci: 3
    else:
        ps1bufs = lambda ci: 3 if ci == 0 else 2
        ps2bufs = lambda ci: 2 if ci == 0 else 1

    with TileContext(nc) as tc:
        with (
            tc.tile_pool(name="weights", bufs=1) as wp,
            tc.tile_pool(name="acts", bufs=1) as ap,
            tc.tile_pool(name="outs", bufs=2) as op,
            tc.tile_pool(name="psum", bufs=2, space="PSUM") as pp,
        ):
            # --- PE warm-up: wide dummy matmuls (512-col moving operand) on
            # a zeroed tile, no DMA deps.  ~530 ns each -> ~3.2 us of PE
            # activity so the HAM clock gate opens right as real work lands.
            dummy = ap.tile([P, 512], mybir.dt.bfloat16, tag="dummy")
            nc.vector.memset(dummy[:], 0.0)
            wps = pp.tile([P, 512], mybir.dt.float32, tag="ps2_0", name="warm_ps",
                          bufs=ps2bufs(0))
            for _ in range(N_WARMUP_MM):
                nc.tensor.matmul(wps[:], dummy[:, :P], dummy[:],
                                 start=True, stop=True)

            NDH = ND // 2
            xts = [ap.tile([P, NDH * C], mybir.dt.bfloat16, tag=f"xt{i}",
                           name=f"xt{i}") for i in range(2)]
            xt_of = lambda d: xts[d // NDH][:, (d % NDH) * C : (d % NDH + 1) * C]
            w1t = wp.tile([P, NF * ND * P], mybir.dt.bfloat16, tag="w1")
            w2t = wp.tile([P, NF * D], mybir.dt.bfloat16, tag="w2")
            mt = wp.tile([P, NF + ND + C], mybir.dt.float32, tag="meta")
            b1t = mt[:, 0:NF]
            b2t = mt[:, NF : NF + ND]
            gt = mt[:, NF + ND : NF + ND + C]
            ht = ap.tile([P, NF * C], mybir.dt.bfloat16, tag="ht")

            # --- DMAs.  Two concurrent HWDGE rings (Sync + Scalar), each
            # draining FIFO.  Only the data needed in the first ~5 us rides
            # the Scalar ring (biases + W1 f0/f1); everything else queues
            # behind x on the Sync ring in need-order, so x streams at
            # near-full HBM bandwidth.
            W1B = ND * P  # columns per W1 f-block

            # DMA plan.  Trigger instructions serialize at ~650 ns on the
            # issuing engine and each transfer's completion semaphore lags
            # the data by ~1-2 us, so: x ships in two halves (the first
            # half's sem lets GEMM1-f0 start accumulating d0..3 early),
            # W1 ships per f-block (sem cadence ~0.7 us vs 2 us consume
            # cadence), W2/gates in large late transfers.  Scalar ring
            # (concurrent with x): b1/b2, W1 f0, W1 f1.
            for i in range(2):
                nc.sync.dma_start(out=xts[i][:],
                                  in_=xT_d[:, i * NDH * C : (i + 1) * NDH * C])
            nc.scalar.dma_start(out=mt[:, : NF + ND], in_=meta_d[:, : NF + ND])
            nc.scalar.dma_start(out=w1t[:, :W1B], in_=w1_d[:, :W1B])
            nc.scalar.dma_start(out=w1t[:, W1B : 2 * W1B],
                                in_=w1_d[:, W1B : 2 * W1B])
            for f in range(2, NF):
                nc.sync.dma_start(out=w1t[:, f * W1B : (f + 1) * W1B],
                                  in_=w1_d[:, f * W1B : (f + 1) * W1B])
            nc.sync.dma_start(out=mt[:, NF + ND :], in_=meta_d[:, NF + ND :])
            NW2 = 2
            w2step = (NF // NW2) * D
            for i in range(NW2):
                nc.sync.dma_start(out=w2t[:, i * w2step : (i + 1) * w2step],
                                  in_=w2_d[:, i * w2step : (i + 1) * w2step])

            # --- GEMM1 + GELU: hT[f*P:(f+1)*P, t].  f0 and f1 accumulate
            # d0..3 as soon as the first x half + their W1 blocks (Scalar
            # ring) land — real work that covers the wait for the second
            # x half's completion semaphore.
            NCHASE = 2
            ps1 = {}
            for f in range(NCHASE):
                ps1[f] = [pp.tile([P, cw], mybir.dt.float32, tag=f"ps1_{ci}",
                                  name=f"ps1_{f}_{ci}", bufs=ps1bufs(ci))
                          for ci, (c0, cw) in enumerate(ck)]
                for d in range(NDH):
                    lhs = w1t[:, f * W1B + d * P : f * W1B + (d + 1) * P]
                    xd = xt_of(d)
                    for ci, (c0, cw) in enumerate(ck):
                        nc.tensor.matmul(ps1[f][ci][:], lhs,
                                         xd[:, c0 : c0 + cw],
                                         start=(d == 0), stop=False)
            for f in range(NF):
                if f < NCHASE:
                    ps, d_lo = ps1[f], NDH
                else:
                    ps = [pp.tile([P, cw], mybir.dt.float32, tag=f"ps1_{ci}",
                                  name=f"ps1_{f}_{ci}", bufs=ps1bufs(ci))
                          for ci, (c0, cw) in enumerate(ck)]
                    d_lo = 0
                for d in range(d_lo, ND):
                    lhs = w1t[:, f * W1B + d * P : f * W1B + (d + 1) * P]
                    xd = xt_of(d)
                    for ci, (c0, cw) in enumerate(ck):
                        nc.tensor.matmul(
                            ps[ci][:],
                            lhs,
                            xd[:, c0 : c0 + cw],
                            start=(d == 0 and d_lo == 0),
                            stop=(d == ND - 1),
                        )
                for ci, (c0, cw) in enumerate(ck):
                    nc.scalar.activation(
                        ht[:, f * C + c0 : f * C + c0 + cw],
                        ps[ci][:],
                        mybir.ActivationFunctionType.Gelu,
                        bias=b1t[:, f : f + 1],
                    )

            # --- GEMM2 + bias + gate: yT[do*P:(do+1)*P, t].
            for do in range(ND):
                ps2 = [pp.tile([P, cw], mybir.dt.float32, tag=f"ps2_{ci}",
                               name=f"ps2_{do}_{ci}", bufs=ps2bufs(ci))
                       for ci, (c0, cw) in enumerate(ck)]
                for f in range(NF):
                    lhs = w2t[:, f * D + do * P : f * D + (do + 1) * P]
                    for ci, (c0, cw) in enumerate(ck):
                        nc.tensor.matmul(
                            ps2[ci][:],
                            lhs,
                            ht[:, f * C + c0 : f * C + c0 + cw],
                            start=(f == 0),
                            stop=(f == NF - 1),
                        )
                ot = op.tile([P, C], mybir.dt.bfloat16, tag="ot",
                             name=f"ot_{do}")
                for ci, (c0, cw) in enumerate(ck):
                    nc.vector.scalar_tensor_tensor(
                        ot[:, c0 : c0 + cw],
                        ps2[ci][:],
                        b2t[:, do : do + 1],
                        gt[:, c0 : c0 + cw],
                        op0=mybir.AluOpType.add,
                        op1=mybir.AluOpType.mult,
                    )
                # Output rides the Scalar ring (idle during GEMM2).  The
                # last do-block ships per-chunk on BOTH rings so the two
                # trigger instructions run in parallel.
                if do == ND - 1 and len(ck) > 1:
                    engs = [nc.scalar, nc.sync]
                    for ci, (c0, cw) in enumerate(ck):
                        engs[ci % 2].dma_start(
                            out=out_d[do * P : (do + 1) * P, c0 : c0 + cw],
                            in_=ot[:, c0 : c0 + cw])
                else:
                    nc.scalar.dma_start(out=out_d[do * P : (do + 1) * P, :],
                                        in_=ot[:])

    nc.compile()
    return nc


def _get_kernel(C):
    if C not in _cache:
        _cache[C] = _build(C)
    return _cache[C]


def _run_spmd(nc, in_maps):
    """run_bass_kernel_spmd, robust to a BASS_TRACE env the image can't
    serve (missing antenv.axon_hooks / artifact upload): install a best-
    effort NTFF hook shim, and on a trace-path failure fall back to an
    untraced run."""
    import os
    from concourse.bass_utils import run_bass_kernel_spmd

    try:
        import antenv.axon_hooks  # noqa: F401
    except ImportError:
        import sys
        import types
        hook = None
        try:
            from trn_agent_boot.trn_boot import _ntff_profile_via_ctypes
            hook = _ntff_profile_via_ctypes("/opt/axon/libaxon_pjrt.so")
        except Exception:
            hook = None
        mod = types.ModuleType("antenv.axon_hooks")
        mod.get_axon_ntff_profile_hook = lambda: hook
        try:
            import antenv
            antenv.axon_hooks = mod
            sys.modules["antenv.axon_hooks"] = mod
        except ImportError:
            pass

    core_ids = list(range(N_CORES))
    try:
        return run_bass_kernel_spmd(nc, in_maps, core_ids)
    except Exception:
        if os.environ.get("BASS_NEVER_TRACE") == "1":
            raise
        os.environ["BASS_NEVER_TRACE"] = "1"
        try:
            return run_bass_kernel_spmd(nc, in_maps, core_ids)
        finally:
            del os.environ["BASS_NEVER_TRACE"]


def kernel(x, anchors, temperature, W1, b1, W2, b2, top_k):

    x = np.asarray(x)
    B, S, D = x.shape
    T = B * S
    E = np.asarray(anchors).shape[0]
    k = int(np.asarray(top_k))

    xf = np.ascontiguousarray(x.reshape(T, D), dtype=np.float32)

    # ---- routing on host (part of the dispatch decision) ----
    xn = xf / np.maximum(np.linalg.norm(xf, axis=-1, keepdims=True), 1e-8)
    an = np.asarray(anchors, dtype=np.float32)
    an = an / np.maximum(np.linalg.norm(an, axis=-1, keepdims=True), 1e-8)
    scores = (xn @ an.T) * abs(float(np.asarray(temperature)))
    scores -= scores.max(axis=-1, keepdims=True)
    probs = np.exp(scores)
    probs /= probs.sum(axis=-1, keepdims=True)
    topi = np.argsort(-probs, axis=-1, kind="stable")[:, :k]  # ties -> low idx
    topv = np.take_along_axis(probs, topi, axis=-1)
    gw = topv / (topv.sum(axis=-1, keepdims=True) + 1e-6)

    rows_per_e = []
    gates_per_e = []
    for e in range(E):
        mask = topi == e
        rows = np.nonzero(mask.any(axis=-1))[0]
        g = np.where(mask[rows], gw[rows], 0.0).sum(axis=-1).astype(np.float32)
        rows_per_e.append(rows)
        gates_per_e.append(g)

    # Hardware capacity: one PSUM bank (512 tokens) per expert.  An expert's
    # overflow beyond capacity (~1.6% of pairs here) is handled in the
    # combine step on the host at fp32 — the standard MoE capacity-overflow
    # policy (tokens can't be dropped: gates are ~0.5).
    CAP = 512
    over_per_e = [(rows_per_e[e][CAP:], gates_per_e[e][CAP:]) for e in range(E)]
    rows_per_e = [r[:CAP] for r in rows_per_e]
    gates_per_e = [g[:CAP] for g in gates_per_e]

    max_count = max(len(r) for r in rows_per_e)
    C = max(64, -(-max_count // 32) * 32)
    nc = _get_kernel(C)

    # ---- per-core shards, pre-arranged into SBUF layouts ----
    x_bf = xf.astype(_BF16)
    ND, NF = D_MODEL // P, D_FF // P
    in_maps = []
    for e in range(N_CORES):
        rows = rows_per_e[e]
        n = len(rows)
        xT = np.zeros((P, ND * C), dtype=_BF16)
        # [P, ND, C] view: xT[p, d, t] = x[rows[t], d*P + p]
        xv = xT.reshape(P, ND, C)
        xv[:, :, :n] = x_bf[rows].reshape(n, ND, P).transpose(2, 1, 0)
        w1 = np.asarray(W1[e], dtype=np.float32).astype(_BF16)
        w1 = np.ascontiguousarray(
            w1.reshape(ND, P, NF, P).transpose(1, 2, 0, 3).reshape(P, NF * ND * P))
        w2 = np.asarray(W2[e], dtype=np.float32).astype(_BF16)
        w2 = np.ascontiguousarray(
            w2.reshape(NF, P, D_MODEL).transpose(1, 0, 2).reshape(P, NF * D_MODEL))
        meta = np.zeros((P, NF + ND + C), dtype=np.float32)
        meta[:, :NF] = np.asarray(b1[e], dtype=np.float32).reshape(NF, P).T
        meta[:, NF : NF + ND] = np.asarray(b2[e], dtype=np.float32).reshape(ND, P).T
        meta[:, NF + ND : NF + ND + n] = gates_per_e[e][None, :]
        in_maps.append({"xT": xT, "w1": w1, "w2": w2, "meta": meta})

    res = _run_spmd(nc, in_maps)
    global last_results
    last_results = res

    # ---- combine (scatter-add the gated expert outputs) ----
    out = np.zeros((T, D_MODEL), dtype=np.float32)
    for e in range(N_CORES):
        rows = rows_per_e[e]
        n = len(rows)
        if n:
            out[rows] += res.results[e]["out"][:, :n].T.astype(np.float32)
        orows, og = over_per_e[e]
        if len(orows):
            z = xf[orows] @ np.asarray(W1[e], dtype=np.float32)
            z += np.asarray(b1[e], dtype=np.float32)
            y = _gelu_exact(z) @ np.asarray(W2[e], dtype=np.float32)
            y += np.asarray(b2[e], dtype=np.float32)
            out[orows] += og[:, None] * y
    return out.reshape(B, S, D_MODEL)


def _gelu_exact(z):
    """0.5*z*(1+erf(z/sqrt(2))) with an |err|<1.5e-7 erf approximation
    (Abramowitz & Stegun 7.1.26)."""
    x = z / np.sqrt(2.0, dtype=np.float32)
    s = np.sign(x)
    a = np.abs(x)
    t = 1.0 / (1.0 + 0.3275911 * a)
    poly = t * (0.254829592 + t * (-0.284496736 + t * (1.421413741
               + t * (-1.453152027 + t * 1.061405429))))
    erf = s * (1.0 - poly * np.exp(-a * a))
    return 0.5 * z * (1.0 + erf.astype(np.float32))


# revision 25
# speedup vs baseline: 1.0482x; 1.0482x over previous
"""MoE (cosine-routed, top-k, 2-layer GELU FFN) on 8 Trainium2 NeuronCores.

Strategy (expert-parallel, per the sharding hint):
  - Host computes the (tiny) routing: cosine scores -> softmax -> top-k ->
    renormalized gate weights. ~34 MFLOP, negligible vs the 34 GFLOP FFN.
  - Tokens are dispatched by top-k expert id: core e receives the tokens
    routed to expert e (padded to capacity C), plus expert e's W1/b1/W2/b2.
  - Each core runs the 2-layer FFN in bf16 (fp32 PSUM accumulation) and
    scales each token's output by its gate weight on-device.
  - Host scatter-adds the (<= top_k) expert contributions per token.

Device layout per core (P = 128 partitions):
  GEMM1: hT[f, t] = sum_d W1[d, f] * xT[d, t]   (W1 tiles stationary)
         -> Gelu(. + b1) on ScalarE, cast to bf16
  GEMM2: yT[d, t] = sum_f W2[f, d] * hT[f, t]   (W2 tiles stationary)
         -> (. + b2) * gate on VectorE, bf16 out

Pipeline (v2):
  - x is DMA'd in 4 d-pair blocks at the head of the Sync HWDGE ring;
    GEMM1's first f-block accumulates d-blocks in arrival order, so the
    PE starts ~2 us after the preamble and never idles long enough for
    the HAM clock gate to re-throttle.
  - W1 f0..f7 (+ meta) stream on the Scalar HWDGE ring concurrently with
    x on the Sync ring; W1 f8..f15 and W2 follow x on the Sync ring.
  - PSUM: GEMM1 chunk-0 triple-buffered so GELU drains can lag ~2 f-blocks
    without stalling the PE; all 8 banks used.
  - Outputs are written bf16 (host combines in fp32), one DMA per
    128-row output block.
"""

import numpy as np
import ml_dtypes

P = 128
D_MODEL = 1024
D_FF = 2048
N_EXPERTS = 8
N_CORES = 8
N_WARMUP_MM = 13

_BF16 = ml_dtypes.bfloat16

_cache: dict = {}
last_results = None  # BassKernelResults of the most recent run (for profiling)


def _chunks(C):
    out = []
    c0 = 0
    while c0 < C:
        cw = min(512, C - c0)
        out.append((c0, cw))
        c0 += cw
    return out


def _build(C):
    """Build + compile the SPMD FFN kernel for capacity C (multiple of 32)."""
    import concourse.mybir as mybir
    from concourse import bacc
    from concourse.tile import TileContext

    D, F = D_MODEL, D_FF
    ND, NF = D // P, F // P

    nc = bacc.Bacc("TRN2", target_bir_lowering=False, debug=False,
                   enable_partition_id=False)

    # Host-pre-arranged layouts (see kernel() for the packing):
    #   xT:  [P, ND*C]    column d*C + t       = x[token t, d*P + part]
    #   w1:  [P, NF*ND*P] column f*ND*P + d*P + j = W1[d*P + part, f*P + j]
    #   w2:  [P, NF*D]    column f*D + j       = W2[f*P + part, j]
    xT_d = nc.dram_tensor("xT", [P, ND * C], mybir.dt.bfloat16, kind="ExternalInput")
    w1_d = nc.dram_tensor("w1", [P, NF * ND * P], mybir.dt.bfloat16,
                          kind="ExternalInput")
    w2_d = nc.dram_tensor("w2", [P, NF * D], mybir.dt.bfloat16, kind="ExternalInput")
    meta_d = nc.dram_tensor("meta", [P, NF + ND + C], mybir.dt.float32,
                            kind="ExternalInput")
    out_d = nc.dram_tensor("out", [D, C], mybir.dt.bfloat16, kind="ExternalOutput")

    ck = _chunks(C)
    # PSUM bank budget (8 banks).  Two-chunk: GEMM1 chunk0 x3, chunk1 x2,
    # GEMM2 chunk0 x2, chunk1 x1.  Single-chunk (C <= 512): GEMM1 x4,
    # GEMM2 x3.  Warm-up accumulator shares the GEMM2 slot (disjoint
    # lifetime).
    if len(ck) == 1:
        ps1bufs = lambda ci: 4
        ps2bufs = lambda ci: 3
    else:
        ps1bufs = lambda ci: 3 if ci == 0 else 2
        ps2bufs = lambda ci: 2 if ci == 0 else 1

    with TileContext(nc) as tc:
        with (
            tc.tile_pool(name="weights", bufs=1) as wp,
            tc.tile_pool(name="acts", bufs=1) as ap,
            tc.tile_pool(name="outs", bufs=2) as op,
            tc.tile_pool(name="psum", bufs=2, space="PSUM") as pp,
        ):
            # --- PE warm-up: wide dummy matmuls (512-col moving operand) on
            # a zeroed tile, no DMA deps.  ~530 ns each -> ~3.2 us of PE
            # activity so the HAM clock gate opens right as real work lands.
            dummy = ap.tile([P, 512], mybir.dt.bfloat16, tag="dummy")
            nc.vector.memset(dummy[:], 0.0)
            wps = pp.tile([P, 512], mybir.dt.float32, tag="ps2_0", name="warm_ps",
                          bufs=ps2bufs(0))
            for _ in range(N_WARMUP_MM):
                nc.tensor.matmul(wps[:], dummy[:, :P], dummy[:],
                                 start=True, stop=True)

            NDH = ND // 2
            xts = [ap.tile([P, NDH * C], mybir.dt.bfloat16, tag=f"xt{i}",
                           name=f"xt{i}") for i in range(2)]
            xt_of = lambda d: xts[d // NDH][:, (d % NDH) * C : (d % NDH + 1) * C]
            w1t = wp.tile([P, NF * ND * P], mybir.dt.bfloat16, tag="w1")
            w2t = wp.tile([P, NF * D], mybir.dt.bfloat16, tag="w2")
            mt = wp.tile([P, NF + ND + C], mybir.dt.float32, tag="meta")
            b1t = mt[:, 0:NF]
            b2t = mt[:, NF : NF + ND]
            gt = mt[:, NF + ND : NF + ND + C]
            ht = ap.tile([P, NF * C], mybir.dt.bfloat16, tag="ht")

            # --- DMAs.  Two concurrent HWDGE rings (Sync + Scalar), each
            # draining FIFO.  Only the data needed in the first ~5 us rides
            # the Scalar ring (biases + W1 f0/f1); everything else queues
            # behind x on the Sync ring in need-order, so x streams at
            # near-full HBM bandwidth.
            W1B = ND * P  # columns per W1 f-block

            # DMA plan.  Trigger instructions serialize at ~650 ns on the
            # issuing engine and each transfer's completion semaphore lags
            # the data by ~1-2 us, so: x ships in two halves (the first
            # half's sem lets GEMM1-f0 start accumulating d0..3 early),
            # W1 ships per f-block (sem cadence ~0.7 us vs 2 us consume
            # cadence), W2/gates in large late transfers.  Scalar ring
            # (concurrent with x): b1/b2, W1 f0, W1 f1.
            for i in range(2):
                nc.sync.dma_start(out=xts[i][:],
                                  in_=xT_d[:, i * NDH * C : (i + 1) * NDH * C])
            nc.scalar.dma_start(out=mt[:, : NF + ND], in_=meta_d[:, : NF + ND])
            nc.scalar.dma_start(out=w1t[:, :W1B], in_=w1_d[:, :W1B])
            nc.scalar.dma_start(out=w1t[:, W1B : 2 * W1B],
                                in_=w1_d[:, W1B : 2 * W1B])
            for f in range(2, NF):
                nc.sync.dma_start(out=w1t[:, f * W1B : (f + 1) * W1B],
                                  in_=w1_d[:, f * W1B : (f + 1) * W1B])
            nc.sync.dma_start(out=mt[:, NF + ND :], in_=meta_d[:, NF + ND :])
            NW2 = 2
            w2step = (NF // NW2) * D
            for i in range(NW2):
                nc.sync.dma_start(out=w2t[:, i * w2step : (i + 1) * w2step],
                                  in_=w2_d[:, i * w2step : (i + 1) * w2step])

            # --- GEMM1 + GELU: hT[f*P:(f+1)*P, t].  The d-loop consumes x
            # halves in arrival order; f0's first MMs start on the first
            # x half's semaphore while the second half streams.
            for f in range(NF):
                ps = [pp.tile([P, cw], mybir.dt.float32, tag=f"ps1_{ci}",
                              name=f"ps1_{f}_{ci}", bufs=ps1bufs(ci))
                      for ci, (c0, cw) in enumerate(ck)]
                for d in range(ND):
                    lhs = w1t[:, f * W1B + d * P : f * W1B + (d + 1) * P]
                    xd = xt_of(d)
                    for ci, (c0, cw) in enumerate(ck):
                        nc.tensor.matmul(
                            ps[ci][:],
                            lhs,
                            xd[:, c0 : c0 + cw],
                            start=(d == 0),
                            stop=(d == ND - 1),
                        )
                for ci, (c0, cw) in enumerate(ck):
                    nc.scalar.activation(
                        ht[:, f * C + c0 : f * C + c0 + cw],
                        ps[ci][:],
                        mybir.ActivationFunctionType.Gelu,
                        bias=b1t[:, f : f + 1],
                    )

            # --- GEMM2 + bias + gate: yT[do*P:(do+1)*P, t].
            for do in range(ND):
                ps2 = [pp.tile([P, cw], mybir.dt.float32, tag=f"ps2_{ci}",
                               name=f"ps2_{do}_{ci}", bufs=ps2bufs(ci))
                       for ci, (c0, cw) in enumerate(ck)]
                for f in range(NF):
                    lhs = w2t[:, f * D + do * P : f * D + (do + 1) * P]
                    for ci, (c0, cw) in enumerate(ck):
                        nc.tensor.matmul(
                            ps2[ci][:],
                            lhs,
                            ht[:, f * C + c0 : f * C + c0 + cw],
                            start=(f == 0),
                            stop=(f == NF - 1),
                        )
                ot = op.tile([P, C], mybir.dt.bfloat16, tag="ot",
                             name=f"ot_{do}")
                for ci, (c0, cw) in enumerate(ck):
                    nc.vector.scalar_tensor_tensor(
                        ot[:, c0 : c0 + cw],
                        ps2[ci][:],
                        b2t[:, do : do + 1],
                        gt[:, c0 : c0 + cw],
                        op0=mybir.AluOpType.add,
                        op1=mybir.AluOpType.mult,
                    )
                # Output rides the Scalar ring (idle during GEMM2).  The
                # last do-block ships per-chunk on BOTH rings so the two
                # trigger instructions run in parallel.
                if do == ND - 1 and len(ck) > 1:
                    engs = [nc.scalar, nc.sync]
                    for ci, (c0, cw) in enumerate(ck):
                        engs[ci % 2].dma_start(
                            out=out_d[do * P : (do + 1) * P, c0 : c0 + cw],
                            in_=ot[:, c0 : c0 + cw])
                else:
                    nc.scalar.dma_start(out=out_d[do * P : (do + 1) * P, :],
                                        in_=ot[:])

    nc.compile()
    return nc


def _get_kernel(C):
    if C not in _cache:
        _cache[C] = _build(C)
    return _cache[C]


def _run_spmd(nc, in_maps):
    """run_bass_kernel_spmd, robust to a BASS_TRACE env the image can't
    serve (missing antenv.axon_hooks / artifact upload): install a best-
    effort NTFF hook shim, and on a trace-path failure fall back to an
    untraced run."""
    import os
    from concourse.bass_utils import run_bass_kernel_spmd

    try:
        import antenv.axon_hooks  # noqa: F401
    except ImportError:
        import sys
        import types
        hook = None
        try:
            from trn_agent_boot.trn_boot import _ntff_profile_via_ctypes
            hook = _ntff_profile_via_ctypes("/opt/axon/libaxon_pjrt.so")
        except Exception:
            hook = None
        mod = types.ModuleType("antenv.axon_hooks")
        mod.get_axon_ntff_profile_hook = lambda: hook
        try:
            import antenv
            antenv.axon_hooks = mod
            sys.modules["antenv.axon_hooks"] = mod
        except ImportError:
            pass

    core_ids = list(range(N_CORES))
    try:
        return run_bass_kernel_spmd(nc, in_maps, core_ids)
    except Exception:
        if os.environ.get("BASS_NEVER_TRACE") == "1":
            raise
        os.environ["BASS_NEVER_TRACE"] = "1"
        try:
            return run_bass_kernel_spmd(nc, in_maps, core_ids)
        finally:
            del os.environ["BASS_NEVER_TRACE"]


def kernel(x, anchors, temperature, W1, b1, W2, b2, top_k):

    x = np.asarray(x)
    B, S, D = x.shape
    T = B * S
    E = np.asarray(anchors).shape[0]
    k = int(np.asarray(top_k))

    xf = np.ascontiguousarray(x.reshape(T, D), dtype=np.float32)

    # ---- routing on host (part of the dispatch decision) ----
    xn = xf / np.maximum(np.linalg.norm(xf, axis=-1, keepdims=True), 1e-8)
    an = np.asarray(anchors, dtype=np.float32)
    an = an / np.maximum(np.linalg.norm(an, axis=-1, keepdims=True), 1e-8)
    scores = (xn @ an.T) * abs(float(np.asarray(temperature)))
    scores -= scores.max(axis=-1, keepdims=True)
    probs = np.exp(scores)
    probs /= probs.sum(axis=-1, keepdims=True)
    topi = np.argsort(-probs, axis=-1, kind="stable")[:, :k]  # ties -> low idx
    topv = np.take_along_axis(probs, topi, axis=-1)
    gw = topv / (topv.sum(axis=-1, keepdims=True) + 1e-6)

    rows_per_e = []
    gates_per_e = []
    for e in range(E):
        mask = topi == e
        rows = np.nonzero(mask.any(axis=-1))[0]
        g = np.where(mask[rows], gw[rows], 0.0).sum(axis=-1).astype(np.float32)
        rows_per_e.append(rows)
        gates_per_e.append(g)

    # Hardware capacity: one PSUM bank (512 tokens) per expert.  An expert's
    # overflow beyond capacity (~1.6% of pairs here) is handled in the
    # combine step on the host at fp32 — the standard MoE capacity-overflow
    # policy (tokens can't be dropped: gates are ~0.5).
    CAP = 512
    over_per_e = [(rows_per_e[e][CAP:], gates_per_e[e][CAP:]) for e in range(E)]
    rows_per_e = [r[:CAP] for r in rows_per_e]
    gates_per_e = [g[:CAP] for g in gates_per_e]

    max_count = max(len(r) for r in rows_per_e)
    C = max(64, -(-max_count // 32) * 32)
    nc = _get_kernel(C)

    # ---- per-core shards, pre-arranged into SBUF layouts ----
    x_bf = xf.astype(_BF16)
    ND, NF = D_MODEL // P, D_FF // P
    in_maps = []
    for e in range(N_CORES):
        rows = rows_per_e[e]
        n = len(rows)
        xT = np.zeros((P, ND * C), dtype=_BF16)
        # [P, ND, C] view: xT[p, d, t] = x[rows[t], d*P + p]
        xv = xT.reshape(P, ND, C)
        xv[:, :, :n] = x_bf[rows].reshape(n, ND, P).transpose(2, 1, 0)
        w1 = np.asarray(W1[e], dtype=np.float32).astype(_BF16)
        w1 = np.ascontiguousarray(
            w1.reshape(ND, P, NF, P).transpose(1, 2, 0, 3).reshape(P, NF * ND * P))
        w2 = np.asarray(W2[e], dtype=np.float32).astype(_BF16)
        w2 = np.ascontiguousarray(
            w2.reshape(NF, P, D_MODEL).transpose(1, 0, 2).reshape(P, NF * D_MODEL))
        meta = np.zeros((P, NF + ND + C), dtype=np.float32)
        meta[:, :NF] = np.asarray(b1[e], dtype=np.float32).reshape(NF, P).T
        meta[:, NF : NF + ND] = np.asarray(b2[e], dtype=np.float32).reshape(ND, P).T
        meta[:, NF + ND : NF + ND + n] = gates_per_e[e][None, :]
        in_maps.append({"xT": xT, "w1": w1, "w2": w2, "meta": meta})

    res = _run_spmd(nc, in_maps)
    global last_results
    last_results = res

    # ---- combine (scatter-add the gated expert outputs) ----
    out = np.zeros((T, D_MODEL), dtype=np.float32)
    for e in range(N_CORES):
        rows = rows_per_e[e]
        n = len(rows)
        if n:
            out[rows] += res.results[e]["out"][:, :n].T.astype(np.float32)
        orows, og = over_per_e[e]
        if len(orows):
            z = xf[orows] @ np.asarray(W1[e], dtype=np.float32)
            z += np.asarray(b1[e], dtype=np.float32)
            y = _gelu_exact(z) @ np.asarray(W2[e], dtype=np.float32)
            y += np.asarray(b2[e], dtype=np.float32)
            out[orows] += og[:, None] * y
    return out.reshape(B, S, D_MODEL)


def _gelu_exact(z):
    """0.5*z*(1+erf(z/sqrt(2))) with an |err|<1.5e-7 erf approximation
    (Abramowitz & Stegun 7.1.26)."""
    x = z / np.sqrt(2.0, dtype=np.float32)
    s = np.sign(x)
    a = np.abs(x)
    t = 1.0 / (1.0 + 0.3275911 * a)
    poly = t * (0.254829592 + t * (-0.284496736 + t * (1.421413741
               + t * (-1.453152027 + t * 1.061405429))))
    erf = s * (1.0 - poly * np.exp(-a * a))
    return 0.5 * z * (1.0 + erf.astype(np.float32))


# revision 27
# speedup vs baseline: 1.0561x; 1.0075x over previous
"""MoE (cosine-routed, top-k, 2-layer GELU FFN) on 8 Trainium2 NeuronCores.

Strategy (expert-parallel, per the sharding hint):
  - Host computes the (tiny) routing: cosine scores -> softmax -> top-k ->
    renormalized gate weights. ~34 MFLOP, negligible vs the 34 GFLOP FFN.
  - Tokens are dispatched by top-k expert id: core e receives the tokens
    routed to expert e (padded to capacity C), plus expert e's W1/b1/W2/b2.
  - Each core runs the 2-layer FFN in bf16 (fp32 PSUM accumulation) and
    scales each token's output by its gate weight on-device.
  - Host scatter-adds the (<= top_k) expert contributions per token.

Device layout per core (P = 128 partitions):
  GEMM1: hT[f, t] = sum_d W1[d, f] * xT[d, t]   (W1 tiles stationary)
         -> Gelu(. + b1) on ScalarE, cast to bf16
  GEMM2: yT[d, t] = sum_f W2[f, d] * hT[f, t]   (W2 tiles stationary)
         -> (. + b2) * gate on VectorE, bf16 out

Pipeline (v2):
  - x is DMA'd in 4 d-pair blocks at the head of the Sync HWDGE ring;
    GEMM1's first f-block accumulates d-blocks in arrival order, so the
    PE starts ~2 us after the preamble and never idles long enough for
    the HAM clock gate to re-throttle.
  - W1 f0..f7 (+ meta) stream on the Scalar HWDGE ring concurrently with
    x on the Sync ring; W1 f8..f15 and W2 follow x on the Sync ring.
  - PSUM: GEMM1 chunk-0 triple-buffered so GELU drains can lag ~2 f-blocks
    without stalling the PE; all 8 banks used.
  - Outputs are written bf16 (host combines in fp32), one DMA per
    128-row output block.
"""

import numpy as np
import ml_dtypes

P = 128
D_MODEL = 1024
D_FF = 2048
N_EXPERTS = 8
N_CORES = 8
N_WARMUP_MM = 13

_BF16 = ml_dtypes.bfloat16

_cache: dict = {}
last_results = None  # BassKernelResults of the most recent run (for profiling)


def _chunks(C):
    out = []
    c0 = 0
    while c0 < C:
        cw = min(512, C - c0)
        out.append((c0, cw))
        c0 += cw
    return out


def _build(C):
    """Build + compile the SPMD FFN kernel for capacity C (multiple of 32)."""
    import concourse.mybir as mybir
    from concourse import bacc
    from concourse.tile import TileContext

    D, F = D_MODEL, D_FF
    ND, NF = D // P, F // P

    nc = bacc.Bacc("TRN2", target_bir_lowering=False, debug=False,
                   enable_partition_id=False)

    # Host-pre-arranged layouts (see kernel() for the packing):
    #   xT:  [P, ND*C]    column d*C + t       = x[token t, d*P + part]
    #   w1:  [P, NF*ND*P] column f*ND*P + d*P + j = W1[d*P + part, f*P + j]
    #   w2:  [P, NF*D]    column f*D + j       = W2[f*P + part, j]
    xT_d = nc.dram_tensor("xT", [P, ND * C], mybir.dt.bfloat16, kind="ExternalInput")
    w1_d = nc.dram_tensor("w1", [P, NF * ND * P], mybir.dt.bfloat16,
                          kind="ExternalInput")
    w2_d = nc.dram_tensor("w2", [P, NF * D], mybir.dt.bfloat16, kind="ExternalInput")
    meta_d = nc.dram_tensor("meta", [P, NF + ND + C], mybir.dt.float32,
                            kind="ExternalInput")
    out_d = nc.dram_tensor("out", [D, C], mybir.dt.bfloat16, kind="ExternalOutput")

    ck = _chunks(C)
    # PSUM bank budget (8 banks).  Two-chunk: GEMM1 chunk0 x3, chunk1 x2,
    # GEMM2 chunk0 x2, chunk1 x1.  Single-chunk (C <= 512): GEMM1 x4,
    # GEMM2 x3.  Warm-up accumulator shares the GEMM2 slot (disjoint
    # lifetime).
    if len(ck) == 1:
        ps1bufs = lambda ci: 4
        ps2bufs = lambda ci: 3
    else:
        ps1bufs = lambda ci: 3 if ci == 0 else 2
        ps2bufs = lambda ci: 2 if ci == 0 else 1

    with TileContext(nc) as tc:
        with (
            tc.tile_pool(name="weights", bufs=1) as wp,
            tc.tile_pool(name="acts", bufs=1) as ap,
            tc.tile_pool(name="outs", bufs=2) as op,
            tc.tile_pool(name="psum", bufs=2, space="PSUM") as pp,
        ):
            # --- PE warm-up: wide dummy matmuls (512-col moving operand) on
            # a zeroed tile, no DMA deps.  ~530 ns each -> ~3.2 us of PE
            # activity so the HAM clock gate opens right as real work lands.
            dummy = ap.tile([P, 512], mybir.dt.bfloat16, tag="dummy")
            nc.vector.memset(dummy[:], 0.0)
            wps = pp.tile([P, 512], mybir.dt.float32, tag="ps2_0", name="warm_ps",
                          bufs=ps2bufs(0))
            for _ in range(N_WARMUP_MM):
                nc.tensor.matmul(wps[:], dummy[:, :P], dummy[:],
                                 start=True, stop=True)

            NXQ = 4  # x ships in NXQ quarters: sems land ~1 us apart so the
            NDQ = ND // NXQ  # f0 d-loop steps quarter-to-quarter, hiding
            xts = [ap.tile([P, NDQ * C], mybir.dt.bfloat16, tag=f"xt{i}",
                           name=f"xt{i}") for i in range(NXQ)]
            xt_of = lambda d: xts[d // NDQ][:, (d % NDQ) * C : (d % NDQ + 1) * C]
            w1t = wp.tile([P, NF * ND * P], mybir.dt.bfloat16, tag="w1")
            w2t = wp.tile([P, NF * D], mybir.dt.bfloat16, tag="w2")
            mt = wp.tile([P, NF + ND + C], mybir.dt.float32, tag="meta")
            b1t = mt[:, 0:NF]
            b2t = mt[:, NF : NF + ND]
            gt = mt[:, NF + ND : NF + ND + C]
            ht = ap.tile([P, NF * C], mybir.dt.bfloat16, tag="ht")

            # --- DMAs.  Two concurrent HWDGE rings (Sync + Scalar), each
            # draining FIFO.  Only the data needed in the first ~5 us rides
            # the Scalar ring (biases + W1 f0/f1); everything else queues
            # behind x on the Sync ring in need-order, so x streams at
            # near-full HBM bandwidth.
            W1B = ND * P  # columns per W1 f-block

            # DMA plan.  Trigger instructions serialize at ~650 ns on the
            # issuing engine and each transfer's completion semaphore lags
            # the data by ~1-2 us, so: x ships in two halves (the first
            # half's sem lets GEMM1-f0 start accumulating d0..3 early),
            # W1 ships per f-block (sem cadence ~0.7 us vs 2 us consume
            # cadence), W2/gates in large late transfers.  Scalar ring
            # (concurrent with x): b1/b2, W1 f0, W1 f1.
            for i in range(NXQ):
                nc.sync.dma_start(out=xts[i][:],
                                  in_=xT_d[:, i * NDQ * C : (i + 1) * NDQ * C])
            nc.scalar.dma_start(out=mt[:, : NF + ND], in_=meta_d[:, : NF + ND])
            nc.scalar.dma_start(out=w1t[:, :W1B], in_=w1_d[:, :W1B])
            nc.scalar.dma_start(out=w1t[:, W1B : 2 * W1B],
                                in_=w1_d[:, W1B : 2 * W1B])
            for f in range(2, NF):
                nc.sync.dma_start(out=w1t[:, f * W1B : (f + 1) * W1B],
                                  in_=w1_d[:, f * W1B : (f + 1) * W1B])
            nc.sync.dma_start(out=mt[:, NF + ND :], in_=meta_d[:, NF + ND :])
            NW2 = 2
            w2step = (NF // NW2) * D
            for i in range(NW2):
                nc.sync.dma_start(out=w2t[:, i * w2step : (i + 1) * w2step],
                                  in_=w2_d[:, i * w2step : (i + 1) * w2step])

            # --- GEMM1 + GELU: hT[f*P:(f+1)*P, t].  The d-loop consumes x
            # halves in arrival order; f0's first MMs start on the first
            # x half's semaphore while the second half streams.
            for f in range(NF):
                ps = [pp.tile([P, cw], mybir.dt.float32, tag=f"ps1_{ci}",
                              name=f"ps1_{f}_{ci}", bufs=ps1bufs(ci))
                      for ci, (c0, cw) in enumerate(ck)]
                for d in range(ND):
                    lhs = w1t[:, f * W1B + d * P : f * W1B + (d + 1) * P]
                    xd = xt_of(d)
                    for ci, (c0, cw) in enumerate(ck):
                        nc.tensor.matmul(
                            ps[ci][:],
                            lhs,
                            xd[:, c0 : c0 + cw],
                            start=(d == 0),
                            stop=(d == ND - 1),
                        )
                for ci, (c0, cw) in enumerate(ck):
                    nc.scalar.activation(
                        ht[:, f * C + c0 : f * C + c0 + cw],
                        ps[ci][:],
                        mybir.ActivationFunctionType.Gelu,
                        bias=b1t[:, f : f + 1],
                    )

            # --- GEMM2 + bias + gate: yT[do*P:(do+1)*P, t].
            for do in range(ND):
                ps2 = [pp.tile([P, cw], mybir.dt.float32, tag=f"ps2_{ci}",
                               name=f"ps2_{do}_{ci}", bufs=ps2bufs(ci))
                       for ci, (c0, cw) in enumerate(ck)]
                for f in range(NF):
                    lhs = w2t[:, f * D + do * P : f * D + (do + 1) * P]
                    for ci, (c0, cw) in enumerate(ck):
                        nc.tensor.matmul(
                            ps2[ci][:],
                            lhs,
                            ht[:, f * C + c0 : f * C + c0 + cw],
                            start=(f == 0),
                            stop=(f == NF - 1),
                        )
                ot = op.tile([P, C], mybir.dt.bfloat16, tag="ot",
                             name=f"ot_{do}")
                for ci, (c0, cw) in enumerate(ck):
                    nc.vector.scalar_tensor_tensor(
                        ot[:, c0 : c0 + cw],
                        ps2[ci][:],
                        b2t[:, do : do + 1],
                        gt[:, c0 : c0 + cw],
                        op0=mybir.AluOpType.add,
                        op1=mybir.AluOpType.mult,
                    )
                # Output rides the Scalar ring (idle during GEMM2).  The
                # last do-block ships per-chunk on BOTH rings so the two
                # trigger instructions run in parallel.
                if do == ND - 1 and len(ck) > 1:
                    engs = [nc.scalar, nc.sync]
                    for ci, (c0, cw) in enumerate(ck):
                        engs[ci % 2].dma_start(
                            out=out_d[do * P : (do + 1) * P, c0 : c0 + cw],
                            in_=ot[:, c0 : c0 + cw])
                else:
                    nc.scalar.dma_start(out=out_d[do * P : (do + 1) * P, :],
                                        in_=ot[:])

    nc.compile()
    return nc


def _get_kernel(C):
    if C not in _cache:
        _cache[C] = _build(C)
    return _cache[C]


def _run_spmd(nc, in_maps):
    """run_bass_kernel_spmd, robust to a BASS_TRACE env the image can't
    serve (missing antenv.axon_hooks / artifact upload): install a best-
    effort NTFF hook shim, and on a trace-path failure fall back to an
    untraced run."""
    import os
    from concourse.bass_utils import run_bass_kernel_spmd

    try:
        import antenv.axon_hooks  # noqa: F401
    except ImportError:
        import sys
        import types
        hook = None
        try:
            from trn_agent_boot.trn_boot import _ntff_profile_via_ctypes
            hook = _ntff_profile_via_ctypes("/opt/axon/libaxon_pjrt.so")
        except Exception:
            hook = None
        mod = types.ModuleType("antenv.axon_hooks")
        mod.get_axon_ntff_profile_hook = lambda: hook
        try:
            import antenv
            antenv.axon_hooks = mod
            sys.modules["antenv.axon_hooks"] = mod
        except ImportError:
            pass

    core_ids = list(range(N_CORES))
    try:
        return run_bass_kernel_spmd(nc, in_maps, core_ids)
    except Exception:
        if os.environ.get("BASS_NEVER_TRACE") == "1":
            raise
        os.environ["BASS_NEVER_TRACE"] = "1"
        try:
            return run_bass_kernel_spmd(nc, in_maps, core_ids)
        finally:
            del os.environ["BASS_NEVER_TRACE"]


def kernel(x, anchors, temperature, W1, b1, W2, b2, top_k):

    x = np.asarray(x)
    B, S, D = x.shape
    T = B * S
    E = np.asarray(anchors).shape[0]
    k = int(np.asarray(top_k))

    xf = np.ascontiguousarray(x.reshape(T, D), dtype=np.float32)

    # ---- routing on host (part of the dispatch decision) ----
    xn = xf / np.maximum(np.linalg.norm(xf, axis=-1, keepdims=True), 1e-8)
    an = np.asarray(anchors, dtype=np.float32)
    an = an / np.maximum(np.linalg.norm(an, axis=-1, keepdims=True), 1e-8)
    scores = (xn @ an.T) * abs(float(np.asarray(temperature)))
    scores -= scores.max(axis=-1, keepdims=True)
    probs = np.exp(scores)
    probs /= probs.sum(axis=-1, keepdims=True)
    topi = np.argsort(-probs, axis=-1, kind="stable")[:, :k]  # ties -> low idx
    topv = np.take_along_axis(probs, topi, axis=-1)
    gw = topv / (topv.sum(axis=-1, keepdims=True) + 1e-6)

    rows_per_e = []
    gates_per_e = []
    for e in range(E):
        mask = topi == e
        rows = np.nonzero(mask.any(axis=-1))[0]
        g = np.where(mask[rows], gw[rows], 0.0).sum(axis=-1).astype(np.float32)
        rows_per_e.append(rows)
        gates_per_e.append(g)

    # Hardware capacity: one PSUM bank (512 tokens) per expert.  An expert's
    # overflow beyond capacity (~1.6% of pairs here) is handled in the
    # combine step on the host at fp32 — the standard MoE capacity-overflow
    # policy (tokens can't be dropped: gates are ~0.5).
    CAP = 512
    over_per_e = [(rows_per_e[e][CAP:], gates_per_e[e][CAP:]) for e in range(E)]
    rows_per_e = [r[:CAP] for r in rows_per_e]
    gates_per_e = [g[:CAP] for g in gates_per_e]

    max_count = max(len(r) for r in rows_per_e)
    C = max(64, -(-max_count // 32) * 32)
    nc = _get_kernel(C)

    # ---- per-core shards, pre-arranged into SBUF layouts ----
    x_bf = xf.astype(_BF16)
    ND, NF = D_MODEL // P, D_FF // P
    in_maps = []
    for e in range(N_CORES):
        rows = rows_per_e[e]
        n = len(rows)
        xT = np.zeros((P, ND * C), dtype=_BF16)
        # [P, ND, C] view: xT[p, d, t] = x[rows[t], d*P + p]
        xv = xT.reshape(P, ND, C)
        xv[:, :, :n] = x_bf[rows].reshape(n, ND, P).transpose(2, 1, 0)
        w1 = np.asarray(W1[e], dtype=np.float32).astype(_BF16)
        w1 = np.ascontiguousarray(
            w1.reshape(ND, P, NF, P).transpose(1, 2, 0, 3).reshape(P, NF * ND * P))
        w2 = np.asarray(W2[e], dtype=np.float32).astype(_BF16)
        w2 = np.ascontiguousarray(
            w2.reshape(NF, P, D_MODEL).transpose(1, 0, 2).reshape(P, NF * D_MODEL))
        meta = np.zeros((P, NF + ND + C), dtype=np.float32)
        meta[:, :NF] = np.asarray(b1[e], dtype=np.float32).reshape(NF, P).T
        meta[:, NF : NF + ND] = np.asarray(b2[e], dtype=np.float32).reshape(ND, P).T
        meta[:, NF + ND : NF + ND + n] = gates_per_e[e][None, :]
        in_maps.append({"xT": xT, "w1": w1, "w2": w2, "meta": meta})

    res = _run_spmd(nc, in_maps)
    global last_results
    last_results = res

    # ---- combine (scatter-add the gated expert outputs) ----
    out = np.zeros((T, D_MODEL), dtype=np.float32)
    for e in range(N_CORES):
        rows = rows_per_e[e]
        n = len(rows)
        if n:
            out[rows] += res.results[e]["out"][:, :n].T.astype(np.float32)
        orows, og = over_per_e[e]
        if len(orows):
            z = xf[orows] @ np.asarray(W1[e], dtype=np.float32)
            z += np.asarray(b1[e], dtype=np.float32)
            y = _gelu_exact(z) @ np.asarray(W2[e], dtype=np.float32)
            y += np.asarray(b2[e], dtype=np.float32)
            out[orows] += og[:, None] * y
    return out.reshape(B, S, D_MODEL)


def _gelu_exact(z):
    """0.5*z*(1+erf(z/sqrt(2))) with an |err|<1.5e-7 erf approximation
    (Abramowitz & Stegun 7.1.26)."""
    x = z / np.sqrt(2.0, dtype=np.float32)
    s = np.sign(x)
    a = np.abs(x)
    t = 1.0 / (1.0 + 0.3275911 * a)
    poly = t * (0.254829592 + t * (-0.284496736 + t * (1.421413741
               + t * (-1.453152027 + t * 1.061405429))))
    erf = s * (1.0 - poly * np.exp(-a * a))
    return 0.5 * z * (1.0 + erf.astype(np.float32))
